# revision 6
# baseline (speedup 1.0000x reference)
"""Multi-head causal attention forward on 8 Trainium2 NeuronCores.

Reference computation (B=2, S=2048, D=1024, H=16, Dh=64):
    q/k/v = einsum("bsm,hmd->bshd", x, W_{Q,K,V}) (+ zero biases)
    scores = q @ k^T / sqrt(Dh), causal mask, softmax
    z = attn @ v
    out = einsum("bqhd,hdm->bqm", z, W_O) + sum_h b_O[h]

Sharding: core c handles batch c//4 and heads 4*(c%4) .. 4*(c%4)+3
(tensor parallel over heads x data parallel over batch). Each core
produces a partial output (sum over its 4 heads); the host sums the 4
partials per batch (the "all-reduce" of the output projection).

v2 schedule: the ACT engine's softmax exp stream is the pacing
resource during attention (1 col/cycle @1.2GHz), so everything else is
arranged around keeping it saturated from ~11us onward:
  - xt is DMA'd in column halves; the prologue mc-streams only the
    ci0-1 chunks of qT0/kT0 plus v sb0-3 so attention (p0,qc0) starts
    as soon as ~2 MiB of x has landed.
  - every other projection (q1/k1 all ci, q0/k0 ci2-3, v sb4-15) and
    the whole output projection run as PE "filler" groups interleaved
    between attention kb steps, sized to the per-block ACT-PE deficit.
  - ACT does exp only (a dummy exp preloads the table during the DMA
    wait); all PSUM evacuation is on DVE; the softmax reciprocal runs
    as DVE recip (bf16) + gpsimd partition broadcast + fused
    scalar_tensor_tensor normalize into zT.
  - scores are emitted one kb ahead of z (psum ring 2) so the PE never
    serializes behind exp; psz is staged to SBUF bf16 at block end to
    free the z psum ring fast.
"""

import os
import sys

import numpy as np

if "/opt/trn_rl_repo" not in sys.path:
    sys.path.insert(0, "/opt/trn_rl_repo")

import concourse.bass as bass
import concourse.bacc as bacc
import concourse.tile as tile
from concourse import mybir
from concourse.alu_op_type import AluOpType
from concourse.bass_utils import run_bass_kernel_spmd

B, S, D, H, Dh = 2, 2048, 1024, 16, 64
HPC = 4          # heads per core
N_CORES = 8
QCH = 512        # q chunk width (one psum bank of fp32)
F32 = mybir.dt.float32
BF16 = mybir.dt.bfloat16


def _build_masks() -> np.ndarray:
    """Lower-triangular [128, 128] bf16 mask for the diagonal score tile:
    element (kp, q) valid iff kp <= q."""
    tri = (np.arange(128)[None, :] >= np.arange(128)[:, None])
    import ml_dtypes
    return np.ascontiguousarray(tri.astype(ml_dtypes.bfloat16))


def _patch_act_tables():
    """Restrict Exp membership to natural_log_exp_and_others so bacc's
    table-load pass emits exactly one load for it."""
    import concourse.bacc as _bacc
    import concourse.hw_specs as _hw

    if getattr(_patch_act_tables, "_done", False):
        return
    orig = _hw.get_activation_tables

    def patched(arch):
        t = {k: set(v) for k, v in orig(arch).items()}
        combined = t.get("natural_log_exp_and_others")
        if combined:
            exp_t = mybir.ActivationFunctionType.Exp
            if exp_t in combined:
                for name, s in t.items():
                    if name != "natural_log_exp_and_others":
                        s.discard(exp_t)
        return t

    _bacc.get_activation_tables = patched
    _patch_act_tables._done = True


def build_bass() -> bass.Bass:
    _patch_act_tables()
    nc = bacc.Bacc("TRN2", target_bir_lowering=False, debug=False)

    xt_d = nc.dram_tensor("xt", [D, S], BF16, kind="ExternalInput")
    wq_d = nc.dram_tensor("wq", [2, D, 128], BF16, kind="ExternalInput")
    wk_d = nc.dram_tensor("wk", [2, D, 128], BF16, kind="ExternalInput")
    wv_d = nc.dram_tensor("wv", [D, HPC * Dh], BF16, kind="ExternalInput")
    wo_d = nc.dram_tensor("wo", [2, 128, D], BF16, kind="ExternalInput")
    out_d = nc.dram_tensor("out", [S, D], BF16, kind="ExternalOutput")
    mask_d = nc.inline_tensor(_build_masks(), "cmask")

    xt = xt_d.ap()
    wq = wq_d.ap()
    wk = wk_d.ap()
    wv = wv_d.ap()
    wo = wo_d.ap()
    out = out_d.ap()
    mask = mask_d.ap()

    EXP = mybir.ActivationFunctionType.Exp

    with tile.TileContext(nc) as tc:
        const_pool = tc.alloc_tile_pool(name="const", bufs=1)
        persist = tc.alloc_tile_pool(name="persist", bufs=1)

        mask_sb = const_pool.tile([128, 128], BF16, name="mask_sb")
        ones64 = const_pool.tile([128, 64], F32, name="ones64")
        nc.vector.memset(ones64, 1.0)
        dummy_in = const_pool.tile([128, 2], F32, name="dummy_in")
        dummy_out = const_pool.tile([128, 2], F32, name="dummy_out")
        nc.vector.memset(dummy_in, 0.0)
        # preload the Exp table on ACT while input DMA is in flight
        nc.scalar.activation(dummy_out, dummy_in, EXP)

        wo_sb = [persist.tile([128, D], BF16, name=f"wo_sb{p}") for p in range(2)]
        qT = [persist.tile([128, S], BF16, name=f"qT{p}") for p in range(2)]
        kT = [persist.tile([128, S], BF16, name=f"kT{p}") for p in range(2)]
        # v' layout: per s-block sb: 4 heads x 65 cols (64 v dims + ones col)
        vp_all = persist.tile([128, 16 * HPC * 65], BF16, name="vp_all")
        zT = [
            [persist.tile([128, QCH], BF16, name=f"zT{p}_{qc}") for qc in range(4)]
            for p in range(2)
        ]
        wq_sb, wk_sb = [], []
        for p in range(2):
            wq_sb.append(persist.tile([128, 8, 128], BF16, name=f"wq_sb{p}"))
            wk_sb.append(persist.tile([128, 8, 128], BF16, name=f"wk_sb{p}"))
        wv_sb = persist.tile([128, 8, HPC * Dh], BF16, name="wv_sb")
        xt_sb = [persist.tile([128, S], BF16, name=f"xt_sb{m}") for m in range(8)]

        # ---- DMA issue order (prologue needs wq0/wk0/wv/mask + xt cols 0:1024) ----
        nc.sync.dma_start(out=wq_sb[0], in_=wq[0].rearrange("(c p) d -> p c d", p=128))
        nc.sync.dma_start(out=wk_sb[0], in_=wk[0].rearrange("(c p) d -> p c d", p=128))
        nc.sync.dma_start(out=wv_sb, in_=wv.rearrange("(c p) d -> p c d", p=128))
        nc.sync.dma_start(out=mask_sb, in_=mask)
        for m in range(8):
            nc.sync.dma_start(
                out=xt_sb[m][:, 0:1024], in_=xt[m * 128 : (m + 1) * 128, 0:1024]
            )
        nc.sync.dma_start(out=wq_sb[1], in_=wq[1].rearrange("(c p) d -> p c d", p=128))
        nc.sync.dma_start(out=wk_sb[1], in_=wk[1].rearrange("(c p) d -> p c d", p=128))
        for m in range(8):
            nc.sync.dma_start(
                out=xt_sb[m][:, 1024:2048], in_=xt[m * 128 : (m + 1) * 128, 1024:2048]
            )
        for p in range(2):
            nc.sync.dma_start(out=wo_sb[p], in_=wo[p])

        # ones columns of v' (one strided f32->bf16 copy over all 64 blocks)
        vcols = vp_all.rearrange("p (n c) -> p n c", c=65)[:, :, 64]
        nc.vector.tensor_copy(vcols, ones64)

        # ---- HAM/pstate warmup: dummy matmuls with no DMA deps ----
        pa = tc.alloc_tile_pool(name="pa", bufs=8, space="PSUM")
        warm = const_pool.tile([128, QCH], BF16, name="warm")
        nc.vector.memset(warm, 1.0)
        for i in range(6):
            wps = pa.tile([128, QCH], F32, tag="a", name=f"wps{i}")
            nc.tensor.matmul(wps, lhsT=warm[:, 0:128], rhs=warm, start=True, stop=True)

        # ---- Prologue: mc-streamed (kT0,qT0) ci0-1 + v sb0-3, 8 psum banks ----
        pk = [pa.tile([128, QCH], F32, tag="a", name=f"pk{ci}") for ci in range(2)]
        pq = [pa.tile([128, QCH], F32, tag="a", name=f"pq{ci}") for ci in range(2)]
        pv = [pa.tile([128, QCH], F32, tag="a", name=f"pv{sb}") for sb in range(4)]
        for mc in range(8):
            st, sp = mc == 0, mc == 7
            for ci in range(2):
                nc.tensor.matmul(
                    pk[ci], lhsT=wk_sb[0][:, mc, :],
                    rhs=xt_sb[mc][:, ci * QCH : (ci + 1) * QCH], start=st, stop=sp,
                )
            for ci in range(2):
                nc.tensor.matmul(
                    pq[ci], lhsT=wq_sb[0][:, mc, :],
                    rhs=xt_sb[mc][:, ci * QCH : (ci + 1) * QCH], start=st, stop=sp,
                )
            for sb in range(4):
                nc.tensor.matmul(
                    pv[sb][:, : HPC * Dh],
                    lhsT=xt_sb[mc][:, sb * 128 : (sb + 1) * 128],
                    rhs=wv_sb[:, mc, :], start=st, stop=sp,
                )

        def evac_v(psv, sb):
            # single strided copy: [4 heads x 64] -> stride-65 blocks
            dstv = vp_all[:, sb * HPC * 65 : (sb + 1) * HPC * 65].rearrange(
                "p (h c) -> p h c", c=65
            )[:, :, 0:64]
            srcv = psv[:, : HPC * Dh].rearrange("p (h c) -> p h c", c=64)
            nc.vector.tensor_copy(dstv, srcv)

        # first-needed first: ci0 evacs unblock attention (p0,qc0)
        nc.scalar.copy(kT[0][:, 0:QCH], pk[0])
        nc.vector.tensor_copy(qT[0][:, 0:QCH], pq[0])
        nc.scalar.copy(kT[0][:, QCH : 2 * QCH], pk[1])
        nc.vector.tensor_copy(qT[0][:, QCH : 2 * QCH], pq[1])
        for sb in range(4):
            evac_v(pv[sb], sb)
        pa.release()

        # ---- Attention-phase psum pools: 4 + 2 + 2 = 8 banks ----
        psum_s = tc.alloc_tile_pool(name="psum_s", bufs=2, space="PSUM")
        psum_z = tc.alloc_tile_pool(name="psum_z", bufs=2, space="PSUM")
        pfill = tc.alloc_tile_pool(name="pfill", bufs=2, space="PSUM")

        pt_pool = tc.alloc_tile_pool(name="pt", bufs=4)
        zs_pool = tc.alloc_tile_pool(name="zs", bufs=2)
        small = tc.alloc_tile_pool(name="small", bufs=2)
        ost = tc.alloc_tile_pool(name="ost", bufs=3)

        # ---- Filler groups (PE work interleaved into attention blocks) ----
        def g_qk(which, pair, ci):
            w_sb = (wq_sb if which == "q" else wk_sb)[pair]
            dst = (qT if which == "q" else kT)[pair]

            def emit():
                pg = pfill.tile([128, QCH], F32, tag="f", name=f"g{which}{pair}{ci}")
                for mc in range(8):
                    nc.tensor.matmul(
                        pg, lhsT=w_sb[:, mc, :],
                        rhs=xt_sb[mc][:, ci * QCH : (ci + 1) * QCH],
                        start=(mc == 0), stop=(mc == 7),
                    )
                nc.vector.tensor_copy(dst[:, ci * QCH : (ci + 1) * QCH], pg)

            return emit

        def g_v(sb):
            def emit():
                pg = pfill.tile([128, QCH], F32, tag="f", name=f"gv{sb}")
                for mc in range(8):
                    nc.tensor.matmul(
                        pg[:, : HPC * Dh],
                        lhsT=xt_sb[mc][:, sb * 128 : (sb + 1) * 128],
                        rhs=wv_sb[:, mc, :], start=(mc == 0), stop=(mc == 7),
                    )
                evac_v(pg, sb)

            return emit

        outsb = {}

        def g_op(qb, mc):
            qc, qi = qb // 4, qb % 4

            def emit():
                pg = pfill.tile([128, QCH], F32, tag="f", name=f"go{qb}_{mc}")
                for p in range(2):
                    nc.tensor.matmul(
                        pg, lhsT=zT[p][qc][:, qi * 128 : (qi + 1) * 128],
                        rhs=wo_sb[p][:, mc * QCH : (mc + 1) * QCH],
                        start=(p == 0), stop=(p == 1),
                    )
                if mc == 0:
                    outsb[qb] = ost.tile([128, D], BF16, tag="o", name=f"ob{qb}")
                nc.vector.tensor_copy(outsb[qb][:, mc * QCH : (mc + 1) * QCH], pg)
                if mc == 1:
                    nc.sync.dma_start(
                        out=out[qb * 128 : (qb + 1) * 128, :], in_=outsb.pop(qb)
                    )

            return emit

        # ---- Deferred softmax-normalize flush ----
        def flush_stage(fl):
            # right after block end: free the z psum ring fast
            psz, zsv = fl["psz"], fl["zs"]
            for hh in (0, 1):
                nc.vector.tensor_copy(zsv[:, hh * QCH : (hh + 1) * QCH], psz[hh])

        def flush_recip(fl):
            zsv = fl["zs"]
            rden = small.tile([1, 2 * QCH], BF16, tag="rd", name=f"rd{fl['id']}")
            with nc.allow_low_precision("softmax denom reciprocal in bf16"):
                nc.vector.reciprocal(rden, zsv[64:65, :])
            rbs = []
            for hh in (0, 1):
                rb = small.tile([64, QCH], BF16, tag=f"rb{hh}", name=f"rb{fl['id']}{hh}")
                nc.gpsimd.partition_broadcast(rb, rden[:, hh * QCH : (hh + 1) * QCH])
                rbs.append(rb)
            fl["rbs"] = rbs

        def flush_mul(fl):
            zsv, (pair, qc) = fl["zs"], fl["dst"]
            for hh in (0, 1):
                nc.vector.scalar_tensor_tensor(
                    zT[pair][qc][hh * 64 : (hh + 1) * 64, :],
                    zsv[0:64, hh * QCH : (hh + 1) * QCH],
                    0.0,
                    fl["rbs"][hh],
                    AluOpType.bypass,
                    AluOpType.mult,
                )

        # ---- Attention block with 1-deep scores pipeline + fillers ----
        pending_flush = []

        def emit_scores(pair, qc, kb, n_kb):
            rel = max(kb - (n_kb - 4), 0) * 128
            pss = psum_s.tile([128, 2 * QCH], F32, tag="s", name=f"ps{pair}{qc}{kb}")
            for hh in (0, 1):
                hoff = hh * 64
                nc.tensor.matmul(
                    pss[:, hh * QCH + rel : (hh + 1) * QCH],
                    lhsT=kT[pair][hoff : hoff + 64, kb * 128 : (kb + 1) * 128],
                    rhs=qT[pair][hoff : hoff + 64, qc * QCH + rel : (qc + 1) * QCH],
                    start=True, stop=True, tile_position=(hoff, 0),
                )
            return pss, rel

        def block(pair, qc, fillers):
            n_kb = 4 * qc + 4
            fq = list(fillers)
            fl = pending_flush.pop(0) if pending_flush else None
            sbuf = [emit_scores(pair, qc, 0, n_kb)]
            psz = [
                psum_z.tile([65, QCH], F32, tag="z", name=f"z{pair}{qc}{hh}")
                for hh in (0, 1)
            ]
            for kb in range(n_kb):
                if kb + 1 < n_kb:
                    sbuf.append(emit_scores(pair, qc, kb + 1, n_kb))
                pss, rel = sbuf.pop(0)
                dt2 = kb - (n_kb - 4)
                pt = pt_pool.tile([128, 2 * QCH], BF16, tag="pt", name=f"pt{pair}{qc}{kb}")
                if rel >= 256:
                    for hh in (0, 1):
                        off = hh * QCH + rel
                        nc.scalar.activation(
                            pt[:, off : hh * QCH + QCH], pss[:, off : hh * QCH + QCH],
                            EXP, scale=0.125,
                        )
                else:
                    nc.scalar.activation(pt, pss, EXP, scale=0.125)
                if dt2 >= 0:
                    for hh in (0, 1):
                        off = hh * QCH + rel
                        nc.vector.tensor_mul(
                            pt[:, off : off + 128], pt[:, off : off + 128], mask_sb
                        )
                if fl is not None and kb == 1:
                    flush_recip(fl)
                if fl is not None and kb == min(3, n_kb - 1):
                    flush_mul(fl)
                for hh in (0, 1):
                    voff = kb * HPC * 65 + (2 * pair + hh) * 65
                    nc.tensor.matmul(
                        psz[hh][:, rel:QCH],
                        lhsT=vp_all[:, voff : voff + 65],
                        rhs=pt[:, hh * QCH + rel : (hh + 1) * QCH],
                        start=(kb == 0), stop=(kb == n_kb - 1),
                    )
                if fq:
                    fq.pop(0)()
            while fq:
                fq.pop(0)()
            zsv = zs_pool.tile([65, 2 * QCH], BF16, tag="zs", name=f"zs{pair}{qc}")
            fl_new = {"psz": psz, "zs": zsv, "dst": (pair, qc), "id": f"{pair}{qc}"}
            flush_stage(fl_new)
            pending_flush.append(fl_new)

        # ---- Filler plan per (pair, qc) block ----
        # Tagged so coverage is verified: every projection chunk must be
        # produced exactly once, before its first consumer block.
        plan_spec = {
            (0, 0): [("k", 1, 0), ("q", 1, 0)],
            (1, 0): [("v", 4), ("v", 5), ("q", 1, 1)],
            (0, 1): [("v", 6), ("v", 7), ("q", 0, 2)],
            (1, 1): [("k", 1, 1), ("q", 1, 2)],
            (0, 2): [("k", 0, 2), ("v", 8), ("v", 9), ("v", 10), ("v", 11)],
            (1, 2): [("k", 1, 2), ("q", 0, 3), ("q", 1, 3), ("o", 0, 0), ("o", 0, 1)],
            (0, 3): [("k", 0, 3), ("v", 12), ("v", 13), ("v", 14), ("v", 15),
                     ("o", 1, 0), ("o", 1, 1), ("o", 2, 0), ("o", 2, 1)],
            (1, 3): [("k", 1, 3), ("o", 3, 0), ("o", 3, 1), ("o", 4, 0), ("o", 4, 1),
                     ("o", 5, 0), ("o", 5, 1), ("o", 6, 0), ("o", 6, 1),
                     ("o", 7, 0), ("o", 7, 1)],
        }
        all_items = [it for items in plan_spec.values() for it in items]
        need = (
            [("q", p, ci) for p in (0, 1) for ci in range(4) if (p, ci) not in ((0, 0), (0, 1))]
            + [("k", p, ci) for p in (0, 1) for ci in range(4) if (p, ci) not in ((0, 0), (0, 1))]
            + [("v", sb) for sb in range(4, 16)]
            + [("o", qb, mc) for qb in range(8) for mc in (0, 1)]
        )
        assert sorted(map(str, all_items)) == sorted(map(str, need)), "filler plan mismatch"

        def to_emit(it):
            if it[0] in ("q", "k"):
                return g_qk(it[0], it[1], it[2])
            if it[0] == "v":
                return g_v(it[1])
            return g_op(it[1], it[2])

        for qc in range(4):
            for pair in range(2):
                block(pair, qc, [to_emit(it) for it in plan_spec[(pair, qc)]])

        # ---- Tail: final flush + remaining out-proj ----
        fl = pending_flush.pop(0)
        flush_recip(fl)
        flush_mul(fl)
        for qb in range(8, 16):
            g_op(qb, 0)()
            g_op(qb, 1)()

        ost.release()
        small.release()
        zs_pool.release()
        pt_pool.release()
        pfill.release()
        psum_z.release()
        psum_s.release()
        persist.release()
        const_pool.release()

    nc.compile()
    return nc


_NC_CACHE: list = []


def _get_nc() -> bass.Bass:
    if not _NC_CACHE:
        _NC_CACHE.append(build_bass())
    return _NC_CACHE[0]


def _core_inputs(x, W_Q, W_K, W_V, W_O, c):
    b = c // HPC
    h0 = HPC * (c % HPC)
    wq = np.stack(
        [W_Q[h0 + 2 * p : h0 + 2 * p + 2].transpose(1, 0, 2).reshape(D, 128) for p in range(2)]
    )
    wk = np.stack(
        [W_K[h0 + 2 * p : h0 + 2 * p + 2].transpose(1, 0, 2).reshape(D, 128) for p in range(2)]
    )
    wv = W_V[h0 : h0 + HPC].transpose(1, 0, 2).reshape(D, HPC * Dh)
    wo = np.stack([W_O[h0 + 2 * p : h0 + 2 * p + 2].reshape(128, D) for p in range(2)])
    import ml_dtypes

    bf = ml_dtypes.bfloat16
    return {
        "xt": np.ascontiguousarray(x[b].T.astype(bf)),
        "wq": np.ascontiguousarray(wq.astype(bf)),
        "wk": np.ascontiguousarray(wk.astype(bf)),
        "wv": np.ascontiguousarray(wv.astype(bf)),
        "wo": np.ascontiguousarray(wo.astype(bf)),
    }


def _ensure_ntff_hook():
    """Install the axon NTFF profile hook if the image's antenv lacks it.

    Only needed for trace=True runs (test harness); the grading path
    (kernel()) never calls this.
    """
    try:
        from antenv.axon_hooks import get_axon_ntff_profile_hook  # noqa: F401
        return
    except ImportError:
        pass
    import types

    import antenv

    holder = {"hook": None}
    mod = types.ModuleType("antenv.axon_hooks")
    mod.set_axon_ntff_profile_hook = lambda h: holder.__setitem__("hook", h)
    mod.get_axon_ntff_profile_hook = lambda: holder["hook"]
    sys.modules["antenv.axon_hooks"] = mod
    antenv.axon_hooks = mod
    try:
        if "/root/.axon_site" not in sys.path:
            sys.path.insert(0, "/root/.axon_site")
        from trn_agent_boot.trn_boot import _ntff_profile_via_ctypes

        so = "/opt/axon/libaxon_pjrt.so"
        if os.path.exists(so):
            mod.set_axon_ntff_profile_hook(_ntff_profile_via_ctypes(so))
    except Exception as e:  # degrade to no tracing
        print(f"NTFF hook install failed: {e}", file=sys.stderr)
    # artifact upload needs S3 creds this container may not have
    import concourse.bass_utils as bu

    bu.upload_artifacts = lambda tmpdir: f"local://{tmpdir}"


def _run(inputs: dict, trace: bool = False):
    x = np.asarray(inputs["x"], np.float32)
    W_Q = np.asarray(inputs["W_Q"], np.float32)
    W_K = np.asarray(inputs["W_K"], np.float32)
    W_V = np.asarray(inputs["W_V"], np.float32)
    W_O = np.asarray(inputs["W_O"], np.float32)
    b_O = np.asarray(inputs["b_O"], np.float32)

    if trace:
        _ensure_ntff_hook()
    nc = _get_nc()
    in_maps = [_core_inputs(x, W_Q, W_K, W_V, W_O, c) for c in range(N_CORES)]
    res = run_bass_kernel_spmd(nc, in_maps, core_ids=list(range(N_CORES)), trace=trace)

    out = np.zeros((B, S, D), np.float32)
    for c in range(N_CORES):
        out[c // HPC] += np.asarray(res.results[c]["out"], dtype=np.float32)
    out += b_O.sum(axis=0)  # b_O is [H, D]; reference adds sum over heads
    return out, res


def kernel(**inputs) -> np.ndarray:
    # b_Q/b_K/b_V are zero in the reference's setup_inputs; the device
    # kernel folds them out. Guard with an exact fallback just in case.
    for name in ("b_Q", "b_K", "b_V"):
        if name in inputs and np.any(np.asarray(inputs[name])):
            return _kernel_numpy_fallback(**inputs)
    out, _ = _run(inputs)
    if not np.isfinite(out).all():
        # transient device flake (observed rarely); one retry clears it
        out, _ = _run(inputs)
    return out


def _kernel_numpy_fallback(x, W_Q, b_Q, W_K, b_K, W_V, W_O, b_V, b_O):
    x = np.asarray(x, np.float32)
    q = np.einsum("bqm,hmd->bqhd", x, W_Q) + b_Q
    k = np.einsum("bkm,hmd->bkhd", x, W_K) + b_K
    v = np.einsum("bkm,hmd->bkhd", x, W_V) + b_V
    s = np.einsum("bqhd,bkhd->bhqk", q, k) / np.sqrt(np.float32(W_Q.shape[-1]))
    causal = np.tril(np.ones((x.shape[1], x.shape[1]), bool))
    s = np.where(causal, s, np.float32(-1e9))
    s = s - s.max(-1, keepdims=True)
    e = np.exp(s)
    attn = e / e.sum(-1, keepdims=True)
    z = np.einsum("bhqk,bkhd->bqhd", attn, v)
    return np.einsum("bqhd,hdm->bqm", z, W_O) + b_O.sum(0)


# revision 16
# speedup vs baseline: 1.0969x; 1.0969x over previous
"""Multi-head causal attention forward on 8 Trainium2 NeuronCores.

Reference computation (B=2, S=2048, D=1024, H=16, Dh=64):
    q/k/v = einsum("bsm,hmd->bshd", x, W_{Q,K,V}) (+ zero biases)
    scores = q @ k^T / sqrt(Dh), causal mask, softmax
    z = attn @ v
    out = einsum("bqhd,hdm->bqm", z, W_O) + sum_h b_O[h]

Sharding: core c handles batch c//4 and heads 4*(c%4) .. 4*(c%4)+3
(tensor parallel over heads x data parallel over batch). Each core
produces a partial output (sum over its 4 heads); the host sums the 4
partials per batch (the "all-reduce" of the output projection).

v2 schedule: the ACT engine's softmax exp stream is the pacing
resource during attention (1 col/cycle @1.2GHz), so everything else is
arranged around keeping it saturated from ~11us onward:
  - xt is DMA'd in column halves; the prologue mc-streams only the
    ci0-1 chunks of qT0/kT0 plus v sb0-3 so attention (p0,qc0) starts
    as soon as ~2 MiB of x has landed.
  - every other projection (q1/k1 all ci, q0/k0 ci2-3, v sb4-15) and
    the whole output projection run as PE "filler" groups interleaved
    between attention kb steps, sized to the per-block ACT-PE deficit.
  - ACT does exp only (a dummy exp preloads the table during the DMA
    wait); all PSUM evacuation is on DVE; the softmax reciprocal runs
    as DVE recip (bf16) + gpsimd partition broadcast + fused
    scalar_tensor_tensor normalize into zT.
  - scores are emitted one kb ahead of z (psum ring 2) so the PE never
    serializes behind exp; psz is staged to SBUF bf16 at block end to
    free the z psum ring fast.
"""

import os
import sys

import numpy as np

if "/opt/trn_rl_repo" not in sys.path:
    sys.path.insert(0, "/opt/trn_rl_repo")

import concourse.bass as bass
import concourse.bacc as bacc
import concourse.tile as tile
from concourse import mybir
from concourse.alu_op_type import AluOpType
from concourse.bass_utils import run_bass_kernel_spmd

B, S, D, H, Dh = 2, 2048, 1024, 16, 64
HPC = 4          # heads per core
N_CORES = 8
QCH = 512        # q chunk width (one psum bank of fp32)
F32 = mybir.dt.float32
BF16 = mybir.dt.bfloat16


def _build_masks() -> np.ndarray:
    """Lower-triangular [128, 128] bf16 mask for the diagonal score tile:
    element (kp, q) valid iff kp <= q."""
    tri = (np.arange(128)[None, :] >= np.arange(128)[:, None])
    import ml_dtypes
    return np.ascontiguousarray(tri.astype(ml_dtypes.bfloat16))


def _patch_act_tables():
    """Restrict Exp membership to natural_log_exp_and_others so bacc's
    table-load pass emits exactly one load for it."""
    import concourse.bacc as _bacc
    import concourse.hw_specs as _hw

    if getattr(_patch_act_tables, "_done", False):
        return
    orig = _hw.get_activation_tables

    def patched(arch):
        t = {k: set(v) for k, v in orig(arch).items()}
        combined = t.get("natural_log_exp_and_others")
        if combined:
            exp_t = mybir.ActivationFunctionType.Exp
            ln_t = next(
                (
                    getattr(mybir.ActivationFunctionType, n)
                    for n in ("Ln", "Log")
                    if hasattr(mybir.ActivationFunctionType, n)
                ),
                None,
            )
            if exp_t in combined and (ln_t is None or ln_t in combined):
                for name, s in t.items():
                    if name != "natural_log_exp_and_others":
                        s.discard(exp_t)
                        if ln_t is not None:
                            s.discard(ln_t)
        return t

    _bacc.get_activation_tables = patched
    _patch_act_tables._done = True


def build_bass() -> bass.Bass:
    _patch_act_tables()
    nc = bacc.Bacc("TRN2", target_bir_lowering=False, debug=False)

    xt_d = nc.dram_tensor("xt", [D, S], BF16, kind="ExternalInput")
    wq_d = nc.dram_tensor("wq", [2, D, 128], BF16, kind="ExternalInput")
    wk_d = nc.dram_tensor("wk", [2, D, 128], BF16, kind="ExternalInput")
    wv_d = nc.dram_tensor("wv", [D, HPC * Dh], BF16, kind="ExternalInput")
    wo_d = nc.dram_tensor("wo", [2, 128, D], BF16, kind="ExternalInput")
    out_d = nc.dram_tensor("out", [S, D], BF16, kind="ExternalOutput")
    mask_d = nc.inline_tensor(_build_masks(), "cmask")

    xt = xt_d.ap()
    wq = wq_d.ap()
    wk = wk_d.ap()
    wv = wv_d.ap()
    wo = wo_d.ap()
    out = out_d.ap()
    mask = mask_d.ap()

    EXP = mybir.ActivationFunctionType.Exp
    LOG = (
        mybir.ActivationFunctionType.Ln
        if hasattr(mybir.ActivationFunctionType, "Ln")
        else mybir.ActivationFunctionType.Log
    )

    with tile.TileContext(nc) as tc:
        const_pool = tc.alloc_tile_pool(name="const", bufs=1)
        persist = tc.alloc_tile_pool(name="persist", bufs=1)

        mask_sb = const_pool.tile([128, 128], BF16, name="mask_sb")
        ones64 = const_pool.tile([128, 64], F32, name="ones64")
        nc.vector.memset(ones64, 1.0)
        dummy_in = const_pool.tile([128, 2], F32, name="dummy_in")
        dummy_out = const_pool.tile([128, 2], F32, name="dummy_out")
        nc.vector.memset(dummy_in, 0.0)
        # preload the Exp table on ACT while input DMA is in flight
        nc.scalar.activation(dummy_out, dummy_in, EXP)

        wo_sb = [persist.tile([128, D], BF16, name=f"wo_sb{p}") for p in range(2)]
        qT = [persist.tile([128, S], BF16, name=f"qT{p}") for p in range(2)]
        kT = [persist.tile([128, S], BF16, name=f"kT{p}") for p in range(2)]
        # v' layout: per s-block sb: 4 heads x 65 cols (64 v dims + ones col)
        vp_all = persist.tile([128, 16 * HPC * 65], BF16, name="vp_all")
        zT = [
            [persist.tile([128, QCH], BF16, name=f"zT{p}_{qc}") for qc in range(4)]
            for p in range(2)
        ]
        wq_sb, wk_sb = [], []
        for p in range(2):
            wq_sb.append(persist.tile([128, 8, 128], BF16, name=f"wq_sb{p}"))
            wk_sb.append(persist.tile([128, 8, 128], BF16, name=f"wk_sb{p}"))
        wv_sb = persist.tile([128, 8, HPC * Dh], BF16, name="wv_sb")
        xt_sb = [persist.tile([128, S], BF16, name=f"xt_sb{m}") for m in range(8)]

        # ---- DMA issue order (prologue needs wq0/wk0/wv/mask + xt cols 0:1024) ----
        nc.sync.dma_start(out=wq_sb[0], in_=wq[0].rearrange("(c p) d -> p c d", p=128))
        nc.sync.dma_start(out=wk_sb[0], in_=wk[0].rearrange("(c p) d -> p c d", p=128))
        nc.sync.dma_start(out=wv_sb, in_=wv.rearrange("(c p) d -> p c d", p=128))
        nc.sync.dma_start(out=mask_sb, in_=mask)
        # the prologue only touches x columns 0:512 -- land those first
        for m in range(8):
            nc.sync.dma_start(
                out=xt_sb[m][:, 0:512], in_=xt[m * 128 : (m + 1) * 128, 0:512]
            )
        for m in range(8):
            nc.sync.dma_start(
                out=xt_sb[m][:, 512:1024], in_=xt[m * 128 : (m + 1) * 128, 512:1024]
            )
        nc.sync.dma_start(out=wq_sb[1], in_=wq[1].rearrange("(c p) d -> p c d", p=128))
        nc.sync.dma_start(out=wk_sb[1], in_=wk[1].rearrange("(c p) d -> p c d", p=128))
        for m in range(8):
            nc.sync.dma_start(
                out=xt_sb[m][:, 1024:2048], in_=xt[m * 128 : (m + 1) * 128, 1024:2048]
            )
        for p in range(2):
            nc.sync.dma_start(out=wo_sb[p], in_=wo[p])

        # ones columns of v' (one strided f32->bf16 copy over all 64 blocks)
        vcols = vp_all.rearrange("p (n c) -> p n c", c=65)[:, :, 64]
        nc.vector.tensor_copy(vcols, ones64)

        # ---- HAM/pstate warmup: dummy matmuls with no DMA deps ----
        pa = tc.alloc_tile_pool(name="pa", bufs=6, space="PSUM")
        warm = const_pool.tile([128, QCH], BF16, name="warm")
        nc.vector.memset(warm, 1.0)
        for i in range(6):
            wps = pa.tile([128, QCH], F32, tag="a", name=f"wps{i}")
            nc.tensor.matmul(wps, lhsT=warm[:, 0:128], rhs=warm, start=True, stop=True)

        # ---- Prologue: mc-streamed (kT0,qT0) ci0 + v sb0-3, 6 psum banks ----
        pk = pa.tile([128, QCH], F32, tag="a", name="pk0")
        pq = pa.tile([128, QCH], F32, tag="a", name="pq0")
        pv = [pa.tile([128, QCH], F32, tag="a", name=f"pv{sb}") for sb in range(4)]
        for mc in range(8):
            st, sp = mc == 0, mc == 7
            nc.tensor.matmul(
                pk, lhsT=wk_sb[0][:, mc, :], rhs=xt_sb[mc][:, 0:QCH], start=st, stop=sp,
            )
            nc.tensor.matmul(
                pq, lhsT=wq_sb[0][:, mc, :], rhs=xt_sb[mc][:, 0:QCH], start=st, stop=sp,
            )
            for sb in range(4):
                nc.tensor.matmul(
                    pv[sb][:, : HPC * Dh],
                    lhsT=xt_sb[mc][:, sb * 128 : (sb + 1) * 128],
                    rhs=wv_sb[:, mc, :], start=st, stop=sp,
                )

        def evac_v(psv, sb):
            # single strided copy: [4 heads x 64] -> stride-65 blocks
            dstv = vp_all[:, sb * HPC * 65 : (sb + 1) * HPC * 65].rearrange(
                "p (h c) -> p h c", c=65
            )[:, :, 0:64]
            srcv = psv[:, : HPC * Dh].rearrange("p (h c) -> p h c", c=64)
            nc.vector.tensor_copy(dstv, srcv)

        # first-needed first: ci0 evacs unblock attention (p0,qc0)
        nc.scalar.copy(kT[0][:, 0:QCH], pk)
        nc.vector.tensor_copy(qT[0][:, 0:QCH], pq)
        for sb in range(4):
            evac_v(pv[sb], sb)
        pa.release()

        # ---- Attention-phase psum pools: 4 + 2 + 2 = 8 banks ----
        psum_s = tc.alloc_tile_pool(name="psum_s", bufs=2, space="PSUM")
        psum_z = tc.alloc_tile_pool(name="psum_z", bufs=2, space="PSUM")
        pfill = tc.alloc_tile_pool(name="pfill", bufs=2, space="PSUM")

        pt_pool = tc.alloc_tile_pool(name="pt", bufs=4)
        zs_pool = tc.alloc_tile_pool(name="zs", bufs=2)
        small = tc.alloc_tile_pool(name="small", bufs=2)
        ost = tc.alloc_tile_pool(name="ost", bufs=3)

        # ---- Filler groups (PE work interleaved into attention blocks) ----
        def g_qk(which, pair, ci):
            w_sb = (wq_sb if which == "q" else wk_sb)[pair]
            dst = (qT if which == "q" else kT)[pair]

            def emit():
                pg = pfill.tile([128, QCH], F32, tag="f", name=f"g{which}{pair}{ci}")
                for mc in range(8):
                    nc.tensor.matmul(
                        pg, lhsT=w_sb[:, mc, :],
                        rhs=xt_sb[mc][:, ci * QCH : (ci + 1) * QCH],
                        start=(mc == 0), stop=(mc == 7),
                    )
                nc.vector.tensor_copy(dst[:, ci * QCH : (ci + 1) * QCH], pg)

            return emit

        def g_v(sb):
            def emit():
                pg = pfill.tile([128, QCH], F32, tag="f", name=f"gv{sb}")
                for mc in range(8):
                    nc.tensor.matmul(
                        pg[:, : HPC * Dh],
                        lhsT=xt_sb[mc][:, sb * 128 : (sb + 1) * 128],
                        rhs=wv_sb[:, mc, :], start=(mc == 0), stop=(mc == 7),
                    )
                evac_v(pg, sb)

            return emit

        outsb = {}

        def g_op(qb, mc):
            qc, qi = qb // 4, qb % 4

            def emit():
                pg = pfill.tile([128, QCH], F32, tag="f", name=f"go{qb}_{mc}")
                for p in range(2):
                    nc.tensor.matmul(
                        pg, lhsT=zT[p][qc][:, qi * 128 : (qi + 1) * 128],
                        rhs=wo_sb[p][:, mc * QCH : (mc + 1) * QCH],
                        start=(p == 0), stop=(p == 1),
                    )
                if mc == 0:
                    outsb[qb] = ost.tile([128, D], BF16, tag="o", name=f"ob{qb}")
                nc.vector.tensor_copy(outsb[qb][:, mc * QCH : (mc + 1) * QCH], pg)
                if mc == 1:
                    nc.sync.dma_start(
                        out=out[qb * 128 : (qb + 1) * 128, :], in_=outsb.pop(qb)
                    )

            return emit

        # ---- Deferred softmax-normalize flush ----
        def flush_stage(fl):
            # right after block end: free the z psum ring fast
            psz, zsv = fl["psz"], fl["zs"]
            for hh in (0, 1):
                nc.vector.tensor_copy(zsv[:, hh * QCH : (hh + 1) * QCH], psz[hh])

        def flush_recip(fl):
            # 1/den via ACT ln + exp(-x): one [1, 2*QCH] pass each, covering
            # both heads; DVE's InstReciprocal is ~6.5us/call -- never use it.
            zsv = fl["zs"]
            lnd = small.tile([1, 2 * QCH], F32, tag="ln", name=f"ln{fl['id']}")
            nc.scalar.activation(lnd, zsv[64:65, :], LOG)
            rden = small.tile([1, 2 * QCH], F32, tag="rd", name=f"rd{fl['id']}")
            nc.scalar.activation(rden, lnd, EXP, scale=-1.0)
            rbs = []
            for hh in (0, 1):
                rb = small.tile([64, QCH], F32, tag=f"rb{hh}", name=f"rb{fl['id']}{hh}")
                nc.gpsimd.partition_broadcast(rb, rden[:, hh * QCH : (hh + 1) * QCH])
                rbs.append(rb)
            fl["rbs"] = rbs

        def flush_mul(fl):
            zsv, (pair, qc) = fl["zs"], fl["dst"]
            for hh in (0, 1):
                nc.vector.tensor_mul(
                    zT[pair][qc][hh * 64 : (hh + 1) * 64, :],
                    zsv[0:64, hh * QCH : (hh + 1) * QCH],
                    fl["rbs"][hh],
                )

        # ---- Attention block with 1-deep scores pipeline + fillers ----
        pending_flush = []

        def emit_scores(pair, qc, kb, n_kb):
            rel = max(kb - (n_kb - 4), 0) * 128
            pss = psum_s.tile([128, 2 * QCH], F32, tag="s", name=f"ps{pair}{qc}{kb}")
            for hh in (0, 1):
                hoff = hh * 64
                nc.tensor.matmul(
                    pss[:, hh * QCH + rel : (hh + 1) * QCH],
                    lhsT=kT[pair][hoff : hoff + 64, kb * 128 : (kb + 1) * 128],
                    rhs=qT[pair][hoff : hoff + 64, qc * QCH + rel : (qc + 1) * QCH],
                    start=True, stop=True, tile_position=(hoff, 0),
                )
            return pss, rel

        def block(pair, qc, fillers):
            n_kb = 4 * qc + 4
            fq = list(fillers)
            fl = pending_flush.pop(0) if pending_flush else None
            sbuf = [emit_scores(pair, qc, 0, n_kb)]
            psz = [
                psum_z.tile([65, QCH], F32, tag="z", name=f"z{pair}{qc}{hh}")
                for hh in (0, 1)
            ]
            for kb in range(n_kb):
                if kb + 1 < n_kb:
                    sbuf.append(emit_scores(pair, qc, kb + 1, n_kb))
                pss, rel = sbuf.pop(0)
                dt2 = kb - (n_kb - 4)
                pt = pt_pool.tile([128, 2 * QCH], BF16, tag="pt", name=f"pt{pair}{qc}{kb}")
                if rel > 0:
                    # one strided activation covering both heads' live columns
                    nc.scalar.activation(
                        pt.rearrange("p (h q) -> p h q", h=2)[:, :, rel:],
                        pss.rearrange("p (h q) -> p h q", h=2)[:, :, rel:],
                        EXP, scale=0.125,
                    )
                else:
                    nc.scalar.activation(pt, pss, EXP, scale=0.125)
                if dt2 >= 0:
                    for hh in (0, 1):
                        off = hh * QCH + rel
                        nc.vector.tensor_mul(
                            pt[:, off : off + 128], pt[:, off : off + 128], mask_sb
                        )
                if fl is not None and kb == 1:
                    flush_recip(fl)
                if fl is not None and kb == min(3, n_kb - 1):
                    flush_mul(fl)
                for hh in (0, 1):
                    voff = kb * HPC * 65 + (2 * pair + hh) * 65
                    nc.tensor.matmul(
                        psz[hh][:, rel:QCH],
                        lhsT=vp_all[:, voff : voff + 65],
                        rhs=pt[:, hh * QCH + rel : (hh + 1) * QCH],
                        start=(kb == 0), stop=(kb == n_kb - 1),
                    )
                if fq:
                    fq.pop(0)()
            while fq:
                fq.pop(0)()
            zsv = zs_pool.tile([65, 2 * QCH], BF16, tag="zs", name=f"zs{pair}{qc}")
            fl_new = {"psz": psz, "zs": zsv, "dst": (pair, qc), "id": f"{pair}{qc}"}
            flush_stage(fl_new)
            pending_flush.append(fl_new)

        # ---- Filler plan per (pair, qc) block ----
        # Tagged so coverage is verified: every projection chunk must be
        # produced exactly once, before its first consumer block.
        plan_spec = {
            (0, 0): [("k", 1, 0), ("q", 1, 0)],
            (1, 0): [("k", 0, 1), ("q", 0, 1)],
            (0, 1): [("v", 4), ("v", 5), ("v", 6), ("v", 7), ("q", 1, 1)],
            (1, 1): [("k", 1, 1), ("q", 0, 2), ("q", 1, 2)],
            (0, 2): [("k", 0, 2), ("v", 8), ("v", 9), ("v", 10), ("v", 11)],
            (1, 2): [("k", 1, 2), ("q", 0, 3), ("q", 1, 3), ("o", 0, 0), ("o", 0, 1)],
            (0, 3): [("k", 0, 3), ("v", 12), ("v", 13), ("v", 14), ("v", 15),
                     ("o", 1, 0), ("o", 1, 1), ("o", 2, 0), ("o", 2, 1)],
            (1, 3): [("k", 1, 3), ("o", 3, 0), ("o", 3, 1), ("o", 4, 0), ("o", 4, 1),
                     ("o", 5, 0), ("o", 5, 1), ("o", 6, 0), ("o", 6, 1),
                     ("o", 7, 0), ("o", 7, 1)],
        }
        all_items = [it for items in plan_spec.values() for it in items]
        need = (
            [("q", p, ci) for p in (0, 1) for ci in range(4) if (p, ci) != (0, 0)]
            + [("k", p, ci) for p in (0, 1) for ci in range(4) if (p, ci) != (0, 0)]
            + [("v", sb) for sb in range(4, 16)]
            + [("o", qb, mc) for qb in range(8) for mc in (0, 1)]
        )
        assert sorted(map(str, all_items)) == sorted(map(str, need)), "filler plan mismatch"

        def to_emit(it):
            if it[0] in ("q", "k"):
                return g_qk(it[0], it[1], it[2])
            if it[0] == "v":
                return g_v(it[1])
            return g_op(it[1], it[2])

        for qc in range(4):
            for pair in range(2):
                block(pair, qc, [to_emit(it) for it in plan_spec[(pair, qc)]])

        # ---- Tail: final flush + remaining out-proj ----
        fl = pending_flush.pop(0)
        flush_recip(fl)
        flush_mul(fl)
        for qb in range(8, 16):
            g_op(qb, 0)()
            g_op(qb, 1)()

        ost.release()
        small.release()
        zs_pool.release()
        pt_pool.release()
        pfill.release()
        psum_z.release()
        psum_s.release()
        persist.release()
        const_pool.release()

    nc.compile()
    return nc


_NC_CACHE: list = []


def _get_nc() -> bass.Bass:
    if not _NC_CACHE:
        _NC_CACHE.append(build_bass())
    return _NC_CACHE[0]


def _core_inputs(x, W_Q, W_K, W_V, W_O, c):
    b = c // HPC
    h0 = HPC * (c % HPC)
    wq = np.stack(
        [W_Q[h0 + 2 * p : h0 + 2 * p + 2].transpose(1, 0, 2).reshape(D, 128) for p in range(2)]
    )
    wk = np.stack(
        [W_K[h0 + 2 * p : h0 + 2 * p + 2].transpose(1, 0, 2).reshape(D, 128) for p in range(2)]
    )
    wv = W_V[h0 : h0 + HPC].transpose(1, 0, 2).reshape(D, HPC * Dh)
    wo = np.stack([W_O[h0 + 2 * p : h0 + 2 * p + 2].reshape(128, D) for p in range(2)])
    import ml_dtypes

    bf = ml_dtypes.bfloat16
    return {
        "xt": np.ascontiguousarray(x[b].T.astype(bf)),
        "wq": np.ascontiguousarray(wq.astype(bf)),
        "wk": np.ascontiguousarray(wk.astype(bf)),
        "wv": np.ascontiguousarray(wv.astype(bf)),
        "wo": np.ascontiguousarray(wo.astype(bf)),
    }


def _ensure_ntff_hook():
    """Install the axon NTFF profile hook if the image's antenv lacks it.

    Only needed for trace=True runs (test harness); the grading path
    (kernel()) never calls this.
    """
    try:
        from antenv.axon_hooks import get_axon_ntff_profile_hook  # noqa: F401
        return
    except ImportError:
        pass
    import types

    import antenv

    holder = {"hook": None}
    mod = types.ModuleType("antenv.axon_hooks")
    mod.set_axon_ntff_profile_hook = lambda h: holder.__setitem__("hook", h)
    mod.get_axon_ntff_profile_hook = lambda: holder["hook"]
    sys.modules["antenv.axon_hooks"] = mod
    antenv.axon_hooks = mod
    try:
        if "/root/.axon_site" not in sys.path:
            sys.path.insert(0, "/root/.axon_site")
        from trn_agent_boot.trn_boot import _ntff_profile_via_ctypes

        so = "/opt/axon/libaxon_pjrt.so"
        if os.path.exists(so):
            mod.set_axon_ntff_profile_hook(_ntff_profile_via_ctypes(so))
    except Exception as e:  # degrade to no tracing
        print(f"NTFF hook install failed: {e}", file=sys.stderr)
    # artifact upload needs S3 creds this container may not have
    import concourse.bass_utils as bu

    bu.upload_artifacts = lambda tmpdir: f"local://{tmpdir}"


def _run(inputs: dict, trace: bool = False):
    x = np.asarray(inputs["x"], np.float32)
    W_Q = np.asarray(inputs["W_Q"], np.float32)
    W_K = np.asarray(inputs["W_K"], np.float32)
    W_V = np.asarray(inputs["W_V"], np.float32)
    W_O = np.asarray(inputs["W_O"], np.float32)
    b_O = np.asarray(inputs["b_O"], np.float32)

    if trace:
        _ensure_ntff_hook()
    nc = _get_nc()
    in_maps = [_core_inputs(x, W_Q, W_K, W_V, W_O, c) for c in range(N_CORES)]
    res = run_bass_kernel_spmd(nc, in_maps, core_ids=list(range(N_CORES)), trace=trace)

    out = np.zeros((B, S, D), np.float32)
    for c in range(N_CORES):
        out[c // HPC] += np.asarray(res.results[c]["out"], dtype=np.float32)
    out += b_O.sum(axis=0)  # b_O is [H, D]; reference adds sum over heads
    return out, res


def kernel(**inputs) -> np.ndarray:
    # b_Q/b_K/b_V are zero in the reference's setup_inputs; the device
    # kernel folds them out. Guard with an exact fallback just in case.
    for name in ("b_Q", "b_K", "b_V"):
        if name in inputs and np.any(np.asarray(inputs[name])):
            return _kernel_numpy_fallback(**inputs)
    out, _ = _run(inputs)
    if not np.isfinite(out).all():
        # transient device flake (observed rarely); one retry clears it
        out, _ = _run(inputs)
    return out


def _kernel_numpy_fallback(x, W_Q, b_Q, W_K, b_K, W_V, W_O, b_V, b_O):
    x = np.asarray(x, np.float32)
    q = np.einsum("bqm,hmd->bqhd", x, W_Q) + b_Q
    k = np.einsum("bkm,hmd->bkhd", x, W_K) + b_K
    v = np.einsum("bkm,hmd->bkhd", x, W_V) + b_V
    s = np.einsum("bqhd,bkhd->bhqk", q, k) / np.sqrt(np.float32(W_Q.shape[-1]))
    causal = np.tril(np.ones((x.shape[1], x.shape[1]), bool))
    s = np.where(causal, s, np.float32(-1e9))
    s = s - s.max(-1, keepdims=True)
    e = np.exp(s)
    attn = e / e.sum(-1, keepdims=True)
    z = np.einsum("bhqk,bkhd->bqhd", attn, v)
    return np.einsum("bqhd,hdm->bqm", z, W_O) + b_O.sum(0)


# revision 21
# speedup vs baseline: 1.2972x; 1.1826x over previous
"""Multi-head causal attention forward on 8 Trainium2 NeuronCores.

Reference computation (B=2, S=2048, D=1024, H=16, Dh=64):
    q/k/v = einsum("bsm,hmd->bshd", x, W_{Q,K,V}) (+ zero biases)
    scores = q @ k^T / sqrt(Dh), causal mask, softmax
    z = attn @ v
    out = einsum("bqhd,hdm->bqm", z, W_O) + sum_h b_O[h]

Sharding: core c handles batch c//4 and heads 4*(c%4) .. 4*(c%4)+3
(tensor parallel over heads x data parallel over batch). Each core
produces a partial output (sum over its 4 heads); the host sums the 4
partials per batch (the "all-reduce" of the output projection).

v2 schedule: the ACT engine's softmax exp stream is the pacing
resource during attention (1 col/cycle @1.2GHz), so everything else is
arranged around keeping it saturated from ~11us onward:
  - xt is DMA'd in column halves; the prologue mc-streams only the
    ci0-1 chunks of qT0/kT0 plus v sb0-3 so attention (p0,qc0) starts
    as soon as ~2 MiB of x has landed.
  - every other projection (q1/k1 all ci, q0/k0 ci2-3, v sb4-15) and
    the whole output projection run as PE "filler" groups interleaved
    between attention kb steps, sized to the per-block ACT-PE deficit.
  - ACT does exp only (a dummy exp preloads the table during the DMA
    wait); all PSUM evacuation is on DVE; the softmax reciprocal runs
    as DVE recip (bf16) + gpsimd partition broadcast + fused
    scalar_tensor_tensor normalize into zT.
  - scores are emitted one kb ahead of z (psum ring 2) so the PE never
    serializes behind exp; psz is staged to SBUF bf16 at block end to
    free the z psum ring fast.
"""

import os
import sys

import numpy as np

if "/opt/trn_rl_repo" not in sys.path:
    sys.path.insert(0, "/opt/trn_rl_repo")

import concourse.bass as bass
import concourse.bacc as bacc
import concourse.tile as tile
from concourse import mybir
from concourse.alu_op_type import AluOpType
from concourse.bass_utils import run_bass_kernel_spmd

B, S, D, H, Dh = 2, 2048, 1024, 16, 64
HPC = 4          # heads per core
N_CORES = 8
QCH = 512        # q chunk width (one psum bank of fp32)
F32 = mybir.dt.float32
BF16 = mybir.dt.bfloat16


def _build_masks() -> np.ndarray:
    """Lower-triangular [128, 128] bf16 mask for the diagonal score tile:
    element (kp, q) valid iff kp <= q."""
    tri = (np.arange(128)[None, :] >= np.arange(128)[:, None])
    import ml_dtypes
    return np.ascontiguousarray(tri.astype(ml_dtypes.bfloat16))


def _patch_act_tables():
    """Restrict Exp membership to natural_log_exp_and_others so bacc's
    table-load pass emits exactly one load for it."""
    import concourse.bacc as _bacc
    import concourse.hw_specs as _hw

    if getattr(_patch_act_tables, "_done", False):
        return
    orig = _hw.get_activation_tables

    def patched(arch):
        t = {k: set(v) for k, v in orig(arch).items()}
        combined = t.get("natural_log_exp_and_others")
        if combined:
            exp_t = mybir.ActivationFunctionType.Exp
            ln_t = next(
                (
                    getattr(mybir.ActivationFunctionType, n)
                    for n in ("Ln", "Log")
                    if hasattr(mybir.ActivationFunctionType, n)
                ),
                None,
            )
            if exp_t in combined and (ln_t is None or ln_t in combined):
                for name, s in t.items():
                    if name != "natural_log_exp_and_others":
                        s.discard(exp_t)
                        if ln_t is not None:
                            s.discard(ln_t)
        return t

    _bacc.get_activation_tables = patched
    _patch_act_tables._done = True


def build_bass() -> bass.Bass:
    _patch_act_tables()
    nc = bacc.Bacc("TRN2", target_bir_lowering=False, debug=False)

    xt_d = nc.dram_tensor("xt", [D, S], BF16, kind="ExternalInput")
    wq_d = nc.dram_tensor("wq", [2, D, 128], BF16, kind="ExternalInput")
    wk_d = nc.dram_tensor("wk", [2, D, 128], BF16, kind="ExternalInput")
    wv_d = nc.dram_tensor("wv", [D, HPC * Dh], BF16, kind="ExternalInput")
    wo_d = nc.dram_tensor("wo", [2, 128, D], BF16, kind="ExternalInput")
    out_d = nc.dram_tensor("out", [S, D], BF16, kind="ExternalOutput")
    mask_d = nc.inline_tensor(_build_masks(), "cmask")

    xt = xt_d.ap()
    wq = wq_d.ap()
    wk = wk_d.ap()
    wv = wv_d.ap()
    wo = wo_d.ap()
    out = out_d.ap()
    mask = mask_d.ap()

    EXP = mybir.ActivationFunctionType.Exp
    LOG = (
        mybir.ActivationFunctionType.Ln
        if hasattr(mybir.ActivationFunctionType, "Ln")
        else mybir.ActivationFunctionType.Log
    )

    with tile.TileContext(nc) as tc:
        const_pool = tc.alloc_tile_pool(name="const", bufs=1)
        persist = tc.alloc_tile_pool(name="persist", bufs=1)

        mask_sb = const_pool.tile([128, 128], BF16, name="mask_sb")
        ones64 = const_pool.tile([128, 64], F32, name="ones64")
        nc.vector.memset(ones64, 1.0)
        dummy_in = const_pool.tile([128, 2], F32, name="dummy_in")
        dummy_out = const_pool.tile([128, 2], F32, name="dummy_out")
        nc.vector.memset(dummy_in, 0.0)
        # preload the Exp table on ACT while input DMA is in flight
        nc.scalar.activation(dummy_out, dummy_in, EXP)

        wo_sb = [persist.tile([128, D], BF16, name=f"wo_sb{p}") for p in range(2)]
        qT = [persist.tile([128, S], BF16, name=f"qT{p}") for p in range(2)]
        kT = [persist.tile([128, S], BF16, name=f"kT{p}") for p in range(2)]
        # v' layout: per s-block sb: 4 heads x 65 cols (64 v dims + ones col)
        vp_all = persist.tile([128, 16 * HPC * 65], BF16, name="vp_all")
        zT = [
            [persist.tile([128, QCH], BF16, name=f"zT{p}_{qc}") for qc in range(4)]
            for p in range(2)
        ]
        wq_sb, wk_sb = [], []
        for p in range(2):
            wq_sb.append(persist.tile([128, 8, 128], BF16, name=f"wq_sb{p}"))
            wk_sb.append(persist.tile([128, 8, 128], BF16, name=f"wk_sb{p}"))
        wv_sb = persist.tile([128, 8, HPC * Dh], BF16, name="wv_sb")
        xt_sb = [persist.tile([128, S], BF16, name=f"xt_sb{m}") for m in range(8)]

        # ---- DMA issue order (prologue needs wq0/wk0/wv/mask + xt cols 0:1024) ----
        nc.sync.dma_start(out=wq_sb[0], in_=wq[0].rearrange("(c p) d -> p c d", p=128))
        nc.sync.dma_start(out=wk_sb[0], in_=wk[0].rearrange("(c p) d -> p c d", p=128))
        nc.sync.dma_start(out=wv_sb, in_=wv.rearrange("(c p) d -> p c d", p=128))
        nc.sync.dma_start(out=mask_sb, in_=mask)
        # the prologue only touches x columns 0:512 -- land those first
        for m in range(8):
            nc.sync.dma_start(
                out=xt_sb[m][:, 0:512], in_=xt[m * 128 : (m + 1) * 128, 0:512]
            )
        for m in range(8):
            nc.sync.dma_start(
                out=xt_sb[m][:, 512:1024], in_=xt[m * 128 : (m + 1) * 128, 512:1024]
            )
        nc.sync.dma_start(out=wq_sb[1], in_=wq[1].rearrange("(c p) d -> p c d", p=128))
        nc.sync.dma_start(out=wk_sb[1], in_=wk[1].rearrange("(c p) d -> p c d", p=128))
        for m in range(8):
            nc.sync.dma_start(
                out=xt_sb[m][:, 1024:2048], in_=xt[m * 128 : (m + 1) * 128, 1024:2048]
            )
        for p in range(2):
            nc.sync.dma_start(out=wo_sb[p], in_=wo[p])

        # ones columns of v' (one strided f32->bf16 copy over all 64 blocks)
        vcols = vp_all.rearrange("p (n c) -> p n c", c=65)[:, :, 64]
        nc.vector.tensor_copy(vcols, ones64)

        # ---- Prologue: mc-streamed (kT0,qT0) ci0 + v sb0-3, 6 psum banks ----
        # (no warmup matmuls: the prologue itself ramps the PE clock; a
        # warmup would only delay the DMA-paced projection stream)
        pa = tc.alloc_tile_pool(name="pa", bufs=6, space="PSUM")
        pk = pa.tile([128, QCH], F32, tag="a", name="pk0")
        pq = pa.tile([128, QCH], F32, tag="a", name="pq0")
        pv = [pa.tile([128, QCH], F32, tag="a", name=f"pv{sb}") for sb in range(4)]
        for mc in range(8):
            st, sp = mc == 0, mc == 7
            nc.tensor.matmul(
                pk, lhsT=wk_sb[0][:, mc, :], rhs=xt_sb[mc][:, 0:QCH], start=st, stop=sp,
            )
            nc.tensor.matmul(
                pq, lhsT=wq_sb[0][:, mc, :], rhs=xt_sb[mc][:, 0:QCH], start=st, stop=sp,
            )
            for sb in range(4):
                nc.tensor.matmul(
                    pv[sb][:, : HPC * Dh],
                    lhsT=xt_sb[mc][:, sb * 128 : (sb + 1) * 128],
                    rhs=wv_sb[:, mc, :], start=st, stop=sp,
                )

        def evac_v(psv, sb):
            # single strided copy: [4 heads x 64] -> stride-65 blocks
            dstv = vp_all[:, sb * HPC * 65 : (sb + 1) * HPC * 65].rearrange(
                "p (h c) -> p h c", c=65
            )[:, :, 0:64]
            srcv = psv[:, : HPC * Dh].rearrange("p (h c) -> p h c", c=64)
            nc.vector.tensor_copy(dstv, srcv)

        # first-needed first: ci0 evacs unblock attention (p0,qc0)
        nc.scalar.copy(kT[0][:, 0:QCH], pk)
        nc.vector.tensor_copy(qT[0][:, 0:QCH], pq)
        for sb in range(4):
            evac_v(pv[sb], sb)
        pa.release()

        # ---- Attention-phase psum pools: 4 + 2 + 2 = 8 banks ----
        psum_s = tc.alloc_tile_pool(name="psum_s", bufs=2, space="PSUM")
        psum_z = tc.alloc_tile_pool(name="psum_z", bufs=2, space="PSUM")
        pfill = tc.alloc_tile_pool(name="pfill", bufs=2, space="PSUM")

        pt_pool = tc.alloc_tile_pool(name="pt", bufs=4)
        zs_pool = tc.alloc_tile_pool(name="zs", bufs=2)
        small = tc.alloc_tile_pool(name="small", bufs=2)
        ost = tc.alloc_tile_pool(name="ost", bufs=3)

        # ---- Filler groups (PE work interleaved into attention blocks) ----
        def g_qk(which, pair, ci):
            w_sb = (wq_sb if which == "q" else wk_sb)[pair]
            dst = (qT if which == "q" else kT)[pair]

            def emit():
                pg = pfill.tile([128, QCH], F32, tag="f", name=f"g{which}{pair}{ci}")
                for mc in range(8):
                    nc.tensor.matmul(
                        pg, lhsT=w_sb[:, mc, :],
                        rhs=xt_sb[mc][:, ci * QCH : (ci + 1) * QCH],
                        start=(mc == 0), stop=(mc == 7),
                    )
                nc.vector.tensor_copy(dst[:, ci * QCH : (ci + 1) * QCH], pg)

            return emit

        def g_v(sb):
            def emit():
                pg = pfill.tile([128, QCH], F32, tag="f", name=f"gv{sb}")
                for mc in range(8):
                    nc.tensor.matmul(
                        pg[:, : HPC * Dh],
                        lhsT=xt_sb[mc][:, sb * 128 : (sb + 1) * 128],
                        rhs=wv_sb[:, mc, :], start=(mc == 0), stop=(mc == 7),
                    )
                evac_v(pg, sb)

            return emit

        outsb = {}

        def g_op(qb, mc, tail=False):
            qc, qi = qb // 4, qb % 4

            def emit():
                pg = pfill.tile([128, QCH], F32, tag="f", name=f"go{qb}_{mc}")
                for p in range(2):
                    nc.tensor.matmul(
                        pg, lhsT=zT[p][qc][:, qi * 128 : (qi + 1) * 128],
                        rhs=wo_sb[p][:, mc * QCH : (mc + 1) * QCH],
                        start=(p == 0), stop=(p == 1),
                    )
                if mc == 0:
                    outsb[qb] = ost.tile([128, D], BF16, tag="o", name=f"ob{qb}")
                dst = outsb[qb][:, mc * QCH : (mc + 1) * QCH]
                if tail and mc == 0:
                    # ACT is idle in the tail; split the evacuation load
                    nc.scalar.copy(dst, pg)
                else:
                    nc.vector.tensor_copy(dst, pg)
                if mc == 1:
                    eng = nc.gpsimd if (tail and qb % 2) else nc.sync
                    eng.dma_start(
                        out=out[qb * 128 : (qb + 1) * 128, :], in_=outsb.pop(qb)
                    )

            return emit

        # ---- Deferred softmax-normalize flush ----
        def flush_stage(fl):
            # right after block end: free the z psum ring fast
            psz, zsv = fl["psz"], fl["zs"]
            for hh in (0, 1):
                nc.vector.tensor_copy(zsv[:, hh * QCH : (hh + 1) * QCH], psz[hh])

        def flush_recip(fl):
            # 1/den via ACT ln + exp(-x): one [1, 2*QCH] pass each, covering
            # both heads; DVE's InstReciprocal is ~6.5us/call -- never use it.
            zsv = fl["zs"]
            lnd = small.tile([1, 2 * QCH], F32, tag="ln", name=f"ln{fl['id']}")
            nc.scalar.activation(lnd, zsv[64:65, :], LOG)
            rden = small.tile([1, 2 * QCH], F32, tag="rd", name=f"rd{fl['id']}")
            nc.scalar.activation(rden, lnd, EXP, scale=-1.0)
            rbs = []
            for hh in (0, 1):
                rb = small.tile([64, QCH], F32, tag=f"rb{hh}", name=f"rb{fl['id']}{hh}")
                nc.gpsimd.partition_broadcast(rb, rden[:, hh * QCH : (hh + 1) * QCH])
                rbs.append(rb)
            fl["rbs"] = rbs

        def flush_mul(fl):
            zsv, (pair, qc) = fl["zs"], fl["dst"]
            for hh in (0, 1):
                nc.vector.tensor_mul(
                    zT[pair][qc][hh * 64 : (hh + 1) * 64, :],
                    zsv[0:64, hh * QCH : (hh + 1) * QCH],
                    fl["rbs"][hh],
                )

        # ---- Attention: flat step list, scores pipelined 2 ahead globally ----
        pending_flush = []

        def emit_scores(pair, qc, kb, n_kb):
            rel = max(kb - (n_kb - 4), 0) * 128
            pss = psum_s.tile([128, 2 * QCH], F32, tag="s", name=f"ps{pair}{qc}{kb}")
            for hh in (0, 1):
                hoff = hh * 64
                nc.tensor.matmul(
                    pss[:, hh * QCH + rel : (hh + 1) * QCH],
                    lhsT=kT[pair][hoff : hoff + 64, kb * 128 : (kb + 1) * 128],
                    rhs=qT[pair][hoff : hoff + 64, qc * QCH + rel : (qc + 1) * QCH],
                    start=True, stop=True, tile_position=(hoff, 0),
                )
            return pss, rel

        # ---- Filler plan per (pair, qc) block ----
        # Tagged so coverage is verified: every projection chunk must be
        # produced exactly once, before its first consumer block.
        plan_spec = {
            (0, 0): [("k", 1, 0), ("q", 1, 0)],
            (1, 0): [("k", 0, 1), ("q", 0, 1)],
            (0, 1): [("v", 4), ("v", 5), ("v", 6), ("v", 7), ("q", 1, 1)],
            (1, 1): [("k", 1, 1), ("q", 0, 2), ("q", 1, 2)],
            (0, 2): [("k", 0, 2), ("v", 8), ("v", 9), ("v", 10), ("v", 11)],
            (1, 2): [("k", 1, 2), ("q", 0, 3), ("q", 1, 3), ("o", 0, 0), ("o", 0, 1)],
            (0, 3): [("k", 0, 3), ("v", 12), ("v", 13), ("v", 14), ("v", 15),
                     ("o", 1, 0), ("o", 1, 1), ("o", 2, 0), ("o", 2, 1)],
            (1, 3): [("k", 1, 3), ("o", 3, 0), ("o", 3, 1), ("o", 4, 0), ("o", 4, 1),
                     ("o", 5, 0), ("o", 5, 1), ("o", 6, 0), ("o", 6, 1),
                     ("o", 7, 0), ("o", 7, 1), ("o", 8, 0), ("o", 8, 1),
                     ("o", 9, 0), ("o", 9, 1)],
        }
        all_items = [it for items in plan_spec.values() for it in items]
        need = (
            [("q", p, ci) for p in (0, 1) for ci in range(4) if (p, ci) != (0, 0)]
            + [("k", p, ci) for p in (0, 1) for ci in range(4) if (p, ci) != (0, 0)]
            + [("v", sb) for sb in range(4, 16)]
            + [("o", qb, mc) for qb in range(10) for mc in (0, 1)]
        )
        assert sorted(map(str, all_items)) == sorted(map(str, need)), "filler plan mismatch"

        def to_emit(it):
            if it[0] in ("q", "k"):
                return g_qk(it[0], it[1], it[2])
            if it[0] == "v":
                return g_v(it[1])
            return g_op(it[1], it[2])

        order = [(pair, qc) for qc in range(4) for pair in range(2)]
        steps = []
        for bi, (pair, qc) in enumerate(order):
            n_kb = 4 * qc + 4
            for kb in range(n_kb):
                steps.append((bi, pair, qc, kb, n_kb))

        s_queue = []          # emitted-but-unconsumed (pss, rel)
        s_next = 0            # index of next step whose scores get emitted
        bstate = {}           # bi -> {psz, fl, fillers}

        def pump_scores(upto):
            nonlocal s_next
            while s_next <= upto and s_next < len(steps):
                _, p_, q_, kb_, nk_ = steps[s_next]
                s_queue.append(emit_scores(p_, q_, kb_, nk_))
                s_next += 1

        for j, (bi, pair, qc, kb, n_kb) in enumerate(steps):
            if kb == 0:
                bstate[bi] = {
                    "psz": [
                        psum_z.tile([65, QCH], F32, tag="z", name=f"z{pair}{qc}{hh}")
                        for hh in (0, 1)
                    ],
                    "fl": pending_flush.pop(0) if pending_flush else None,
                    "fillers": [to_emit(it) for it in plan_spec[(pair, qc)]],
                }
            st = bstate[bi]
            pump_scores(j + 1)
            pss, rel = s_queue.pop(0)
            dt2 = kb - (n_kb - 4)
            pt = pt_pool.tile([128, 2 * QCH], BF16, tag="pt", name=f"pt{pair}{qc}{kb}")
            if rel >= 256:
                for hh in (0, 1):
                    off = hh * QCH + rel
                    nc.scalar.activation(
                        pt[:, off : hh * QCH + QCH], pss[:, off : hh * QCH + QCH],
                        EXP, scale=0.125,
                    )
            else:
                nc.scalar.activation(pt, pss, EXP, scale=0.125)
            if dt2 >= 0:
                for hh in (0, 1):
                    off = hh * QCH + rel
                    nc.vector.tensor_mul(
                        pt[:, off : off + 128], pt[:, off : off + 128], mask_sb
                    )
            if st["fl"] is not None and kb == 1:
                flush_recip(st["fl"])
            if st["fl"] is not None and kb == min(3, n_kb - 1):
                flush_mul(st["fl"])
            for hh in (0, 1):
                voff = kb * HPC * 65 + (2 * pair + hh) * 65
                nc.tensor.matmul(
                    st["psz"][hh][:, rel:QCH],
                    lhsT=vp_all[:, voff : voff + 65],
                    rhs=pt[:, hh * QCH + rel : (hh + 1) * QCH],
                    start=(kb == 0), stop=(kb == n_kb - 1),
                )
            if st["fillers"]:
                st["fillers"].pop(0)()
            if kb == n_kb - 1:
                while st["fillers"]:
                    st["fillers"].pop(0)()
                if bi < len(order) - 1:
                    zsv = zs_pool.tile(
                        [65, 2 * QCH], BF16, tag="zs", name=f"zs{pair}{qc}"
                    )
                    fl_new = {
                        "psz": st["psz"], "zs": zsv,
                        "dst": (pair, qc), "id": f"{pair}{qc}",
                    }
                    flush_stage(fl_new)
                    pending_flush.append(fl_new)

        # ---- Tail: last block's flush straight from PSUM, then out-proj ----
        psz = bstate[len(order) - 1]["psz"]
        lnd = small.tile([1, 2 * QCH], F32, tag="ln", name="lnT")
        rden = small.tile([1, 2 * QCH], F32, tag="rd", name="rdT")
        for hh in (0, 1):
            nc.scalar.activation(
                lnd[:, hh * QCH : (hh + 1) * QCH], psz[hh][64:65, :], LOG
            )
        nc.scalar.activation(rden, lnd, EXP, scale=-1.0)
        for hh in (0, 1):
            rb = small.tile([64, QCH], F32, tag=f"rb{hh}", name=f"rbT{hh}")
            nc.gpsimd.partition_broadcast(rb, rden[:, hh * QCH : (hh + 1) * QCH])
            nc.vector.tensor_mul(zT[1][3][hh * 64 : (hh + 1) * 64, :], psz[hh][0:64, :], rb)
        for qb in range(10, 16):
            g_op(qb, 0, tail=True)()
            g_op(qb, 1, tail=True)()

        ost.release()
        small.release()
        zs_pool.release()
        pt_pool.release()
        pfill.release()
        psum_z.release()
        psum_s.release()
        persist.release()
        const_pool.release()

    nc.compile()
    return nc


_NC_CACHE: list = []


def _get_nc() -> bass.Bass:
    if not _NC_CACHE:
        _NC_CACHE.append(build_bass())
    return _NC_CACHE[0]


def _core_inputs(x, W_Q, W_K, W_V, W_O, c):
    b = c // HPC
    h0 = HPC * (c % HPC)
    wq = np.stack(
        [W_Q[h0 + 2 * p : h0 + 2 * p + 2].transpose(1, 0, 2).reshape(D, 128) for p in range(2)]
    )
    wk = np.stack(
        [W_K[h0 + 2 * p : h0 + 2 * p + 2].transpose(1, 0, 2).reshape(D, 128) for p in range(2)]
    )
    wv = W_V[h0 : h0 + HPC].transpose(1, 0, 2).reshape(D, HPC * Dh)
    wo = np.stack([W_O[h0 + 2 * p : h0 + 2 * p + 2].reshape(128, D) for p in range(2)])
    import ml_dtypes

    bf = ml_dtypes.bfloat16
    return {
        "xt": np.ascontiguousarray(x[b].T.astype(bf)),
        "wq": np.ascontiguousarray(wq.astype(bf)),
        "wk": np.ascontiguousarray(wk.astype(bf)),
        "wv": np.ascontiguousarray(wv.astype(bf)),
        "wo": np.ascontiguousarray(wo.astype(bf)),
    }


def _ensure_ntff_hook():
    """Install the axon NTFF profile hook if the image's antenv lacks it.

    Only needed for trace=True runs (test harness); the grading path
    (kernel()) never calls this.
    """
    try:
        from antenv.axon_hooks import get_axon_ntff_profile_hook  # noqa: F401
        return
    except ImportError:
        pass
    import types

    import antenv

    holder = {"hook": None}
    mod = types.ModuleType("antenv.axon_hooks")
    mod.set_axon_ntff_profile_hook = lambda h: holder.__setitem__("hook", h)
    mod.get_axon_ntff_profile_hook = lambda: holder["hook"]
    sys.modules["antenv.axon_hooks"] = mod
    antenv.axon_hooks = mod
    try:
        if "/root/.axon_site" not in sys.path:
            sys.path.insert(0, "/root/.axon_site")
        from trn_agent_boot.trn_boot import _ntff_profile_via_ctypes

        so = "/opt/axon/libaxon_pjrt.so"
        if os.path.exists(so):
            mod.set_axon_ntff_profile_hook(_ntff_profile_via_ctypes(so))
    except Exception as e:  # degrade to no tracing
        print(f"NTFF hook install failed: {e}", file=sys.stderr)
    # artifact upload needs S3 creds this container may not have
    import concourse.bass_utils as bu

    bu.upload_artifacts = lambda tmpdir: f"local://{tmpdir}"


def _run(inputs: dict, trace: bool = False):
    x = np.asarray(inputs["x"], np.float32)
    W_Q = np.asarray(inputs["W_Q"], np.float32)
    W_K = np.asarray(inputs["W_K"], np.float32)
    W_V = np.asarray(inputs["W_V"], np.float32)
    W_O = np.asarray(inputs["W_O"], np.float32)
    b_O = np.asarray(inputs["b_O"], np.float32)

    if trace:
        _ensure_ntff_hook()
    nc = _get_nc()
    in_maps = [_core_inputs(x, W_Q, W_K, W_V, W_O, c) for c in range(N_CORES)]
    res = run_bass_kernel_spmd(nc, in_maps, core_ids=list(range(N_CORES)), trace=trace)

    out = np.zeros((B, S, D), np.float32)
    for c in range(N_CORES):
        out[c // HPC] += np.asarray(res.results[c]["out"], dtype=np.float32)
    out += b_O.sum(axis=0)  # b_O is [H, D]; reference adds sum over heads
    return out, res


def kernel(**inputs) -> np.ndarray:
    # b_Q/b_K/b_V are zero in the reference's setup_inputs; the device
    # kernel folds them out. Guard with an exact fallback just in case.
    for name in ("b_Q", "b_K", "b_V"):
        if name in inputs and np.any(np.asarray(inputs[name])):
            return _kernel_numpy_fallback(**inputs)
    out, _ = _run(inputs)
    if not np.isfinite(out).all():
        # transient device flake (observed rarely); one retry clears it
        out, _ = _run(inputs)
    return out


def _kernel_numpy_fallback(x, W_Q, b_Q, W_K, b_K, W_V, W_O, b_V, b_O):
    x = np.asarray(x, np.float32)
    q = np.einsum("bqm,hmd->bqhd", x, W_Q) + b_Q
    k = np.einsum("bkm,hmd->bkhd", x, W_K) + b_K
    v = np.einsum("bkm,hmd->bkhd", x, W_V) + b_V
    s = np.einsum("bqhd,bkhd->bhqk", q, k) / np.sqrt(np.float32(W_Q.shape[-1]))
    causal = np.tril(np.ones((x.shape[1], x.shape[1]), bool))
    s = np.where(causal, s, np.float32(-1e9))
    s = s - s.max(-1, keepdims=True)
    e = np.exp(s)
    attn = e / e.sum(-1, keepdims=True)
    z = np.einsum("bhqk,bkhd->bqhd", attn, v)
    return np.einsum("bqhd,hdm->bqm", z, W_O) + b_O.sum(0)
